# revision 6
# baseline (speedup 1.0000x reference)
"""Multi-head causal self-attention (N=4, L=2048, E=1024, H=16) on 8 NeuronCores.

Sharding: core c handles batch b = c//2 and head-group g = c%2 (8 heads,
E-slice of 512). Each core computes its QKV projection slice, causal
attention for its 8 heads, and a partial out-projection (E-contraction over
its 512-slice). Host sums the two partials per batch (bias added on g=0).

On-chip layout (per core):
  qT/kT: [e_out(512) x L] transposed activations (4 tiles of [128, 2048])
  v:     [L x e_out] natural layout, per l-block tiles [128, 8 heads, 65]
         (65th column = 1.0 -> the ones column makes the attention matmul
          also produce the softmax denominator as output row 64)
  scores are computed transposed: s^T[l_k, l_q] = k^T.T @ q^T, so the
  av matmul (lhsT = v tile, rhs = exp(s^T)) needs no transposes at all.
  Softmax uses no max-subtraction (scores*scale is O(1) by construction),
  masking is multiplicative post-exp on the block-diagonal tiles only.
All matmuls run as float32r (single-pass, 1 cyc/row at N=512).
"""

import numpy as np

import concourse.bacc as bacc
import concourse.mybir as mybir
import concourse.tile as tile
from concourse import bass_utils

F32 = mybir.dt.float32
F32R = mybir.dt.float32r
AF = mybir.ActivationFunctionType

N, L, E = 4, 2048, 1024
H, EH = 16, 64
NCORES = 8
ES = 512          # e-slice per core (8 heads x 64)
SCALE = 1.0 / np.sqrt(EH)

_CACHE = {}


def _build():
    nc = bacc.Bacc("TRN2", target_bir_lowering=False, debug=False,
                   num_devices=NCORES)
    xq = nc.dram_tensor("xq", (E, L), F32, kind="ExternalInput").ap()
    xk = nc.dram_tensor("xk", (E, L), F32, kind="ExternalInput").ap()
    xv = nc.dram_tensor("xv", (E, L), F32, kind="ExternalInput").ap()
    wq = nc.dram_tensor("wq", (E, ES), F32, kind="ExternalInput").ap()
    wk = nc.dram_tensor("wk", (E, ES), F32, kind="ExternalInput").ap()
    wv = nc.dram_tensor("wv", (E, ES), F32, kind="ExternalInput").ap()
    wo = nc.dram_tensor("wo", (ES, E), F32, kind="ExternalInput").ap()
    bq = nc.dram_tensor("bq", (128, 4), F32, kind="ExternalInput").ap()
    bk = nc.dram_tensor("bk", (128, 4), F32, kind="ExternalInput").ap()
    bv = nc.dram_tensor("bv", (1, ES), F32, kind="ExternalInput").ap()
    bo = nc.dram_tensor("bo", (1, E), F32, kind="ExternalInput").ap()
    msk = nc.dram_tensor("msk", (4, 128, 512), F32, kind="ExternalInput").ap()
    y = nc.dram_tensor("y", (L, E), F32, kind="ExternalOutput").ap()

    with tile.TileContext(nc) as tc:
        with tc.tile_pool(name="const", bufs=1) as cpool, \
             tc.tile_pool(name="qt", bufs=4) as qtpool, \
             tc.tile_pool(name="kt", bufs=4) as ktpool, \
             tc.tile_pool(name="vp", bufs=16) as vpool, \
             tc.tile_pool(name="wo", bufs=4) as wopool:

            ones_st = cpool.tile([1, 128], F32)
            nc.vector.memset(ones_st, 1.0)
            ones = cpool.tile([1, 128], F32R)
            nc.vector.tensor_copy(ones, ones_st)
            vcol = cpool.tile([128, 8], F32)
            nc.vector.memset(vcol, 1.0)
            bq_sb = cpool.tile([128, 4], F32)
            bk_sb = cpool.tile([128, 4], F32)
            bv_sb = cpool.tile([1, ES], F32R)
            bo_sb = cpool.tile([1, E], F32R)
            mask_sb = cpool.tile([128, 4, 512], F32)
            nc.sync.dma_start(out=bq_sb, in_=bq)
            nc.sync.dma_start(out=bk_sb, in_=bk)
            nc.sync.dma_start(out=bv_sb, in_=bv.bitcast(F32R))
            nc.sync.dma_start(out=bo_sb, in_=bo.bitcast(F32R))
            nc.sync.dma_start(out=mask_sb,
                              in_=msk.rearrange("m p q -> p m q"))

            wo_t = []
            for pr in range(4):
                t = wopool.tile([128, E], F32R, tag="wo", name=f"wo{pr}")
                nc.sync.dma_start(
                    out=t, in_=wo[pr * 128:(pr + 1) * 128, :].bitcast(F32R))
                wo_t.append(t)

            qt = [qtpool.tile([128, L], F32R, tag="qt", name=f"qt{i}")
                  for i in range(4)]
            kt = [ktpool.tile([128, L], F32R, tag="kt", name=f"kt{i}")
                  for i in range(4)]
            vts = [vpool.tile([128, 8, 65], F32R, tag="v", name=f"v{i}")
                   for i in range(16)]

            # ---------------- Phase 1: projections ----------------
            with tc.tile_pool(name="wtile", bufs=8) as wpool, \
                 tc.tile_pool(name="xs", bufs=2) as xpool, \
                 tc.tile_pool(name="ps1", bufs=4, space="PSUM") as ps1:

                def load_w(w_dram):
                    ts = []
                    for ko in range(8):
                        t = wpool.tile([128, ES], F32R, tag="w", name=f"w{ko}")
                        nc.sync.dma_start(
                            out=t,
                            in_=w_dram[ko * 128:(ko + 1) * 128, :].bitcast(F32R))
                        ts.append(t)
                    return ts

                def proj_qk(x_dram, w_tiles, bias_sb, out_tiles):
                    for lb in range(4):
                        xt = xpool.tile([128, 8, 512], F32R, tag="x", name="xt")
                        nc.sync.dma_start(
                            out=xt,
                            in_=x_dram.rearrange("(ko ki) l -> ki ko l", ki=128)
                            [:, :, lb * 512:(lb + 1) * 512].bitcast(F32R))
                        for eo in range(4):
                            ps = ps1.tile([128, 512], F32, tag="ps1", name="ps")
                            for ko in range(8):
                                nc.tensor.matmul(
                                    ps,
                                    w_tiles[ko][:, eo * 128:(eo + 1) * 128],
                                    xt[:, ko, :],
                                    start=(ko == 0), stop=(ko == 7))
                            nc.scalar.activation(
                                out_tiles[eo][:, lb * 512:(lb + 1) * 512],
                                ps, AF.Identity,
                                bias=bias_sb[:, eo:eo + 1], scale=1.0)

                wq_t = load_w(wq)
                proj_qk(xq, wq_t, bq_sb, qt)
                wk_t = load_w(wk)
                proj_qk(xk, wk_t, bk_sb, kt)

                wv_t = load_w(wv)
                for lb in range(4):
                    xt = xpool.tile([128, 8, 512], F32R, tag="x", name="xt")
                    nc.sync.dma_start(
                        out=xt,
                        in_=xv.rearrange("(ko ki) l -> ki ko l", ki=128)
                        [:, :, lb * 512:(lb + 1) * 512].bitcast(F32R))
                    for i in range(4):
                        lv = lb * 4 + i
                        ps = ps1.tile([128, 512], F32, tag="ps1", name="ps")
                        for ko in range(8):
                            nc.tensor.matmul(
                                ps, xt[:, ko, i * 128:(i + 1) * 128], wv_t[ko],
                                start=(ko == 0), stop=False)
                        nc.tensor.matmul(ps, ones, bv_sb,
                                         start=False, stop=True)
                        nc.scalar.copy(
                            vts[lv][:, :, 0:64],
                            ps[:, :].rearrange("p (h e) -> p h e", e=64))
                        nc.vector.tensor_copy(vts[lv][:, :, 64], vcol)

            # ---------------- Phase 2+3: attention + out-proj ----------------
            with tc.tile_pool(name="sp", bufs=2, space="PSUM") as sp, \
                 tc.tile_pool(name="op", bufs=2, space="PSUM") as op, \
                 tc.tile_pool(name="bp", bufs=1, space="PSUM") as bp, \
                 tc.tile_pool(name="fp", bufs=1, space="PSUM") as fp, \
                 tc.tile_pool(name="pp", bufs=3) as ppool, \
                 tc.tile_pool(name="tp", bufs=2) as tpool, \
                 tc.tile_pool(name="rp", bufs=2) as rpool, \
                 tc.tile_pool(name="rb", bufs=2) as rbpool, \
                 tc.tile_pool(name="nm", bufs=2) as nmpool, \
                 tc.tile_pool(name="a2", bufs=8) as a2pool, \
                 tc.tile_pool(name="ot", bufs=2) as otpool:

                for jq in range(4):
                    at2 = [a2pool.tile([128, 512], F32R, tag="a2", name=f"a2_{i}")
                           for i in range(4)]
                    for h in range(8):
                        t, po = h // 2, (h % 2) * 64
                        pso = op.tile([65, 512], F32, tag="op")
                        nkb = 4 * (jq + 1)
                        for g in range(2 * (jq + 1)):
                            pss = sp.tile([128, 2, 512], F32, tag="sp")
                            for i in range(2):
                                kb = 2 * g + i
                                nc.tensor.matmul(
                                    pss[:, i, :],
                                    kt[t][po:po + 64,
                                          kb * 128:(kb + 1) * 128],
                                    qt[t][po:po + 64,
                                          jq * 512:(jq + 1) * 512],
                                    start=True, stop=True)
                            p2 = ppool.tile([128, 2, 512], F32R, tag="p")
                            if g >= 2 * jq:   # block-diagonal: needs mask
                                tmp = tpool.tile([128, 2, 512], F32, tag="tmp")
                                nc.scalar.activation(tmp, pss, AF.Exp,
                                                     scale=float(SCALE))
                                mi = 2 * (g - 2 * jq)
                                nc.vector.tensor_mul(
                                    p2, tmp, mask_sb[:, mi:mi + 2, :])
                            else:
                                nc.scalar.activation(p2, pss, AF.Exp,
                                                     scale=float(SCALE))
                            for i in range(2):
                                kb = 2 * g + i
                                nc.tensor.matmul(
                                    pso, vts[kb][:, h, :], p2[:, i, :],
                                    start=(kb == 0), stop=(kb == nkb - 1))
                        # normalize by the denominator (row 64 of pso)
                        rc = rpool.tile([1, 512], F32R, tag="rc")
                        with nc.allow_low_precision(
                                reason="f32r rounding of softmax recip"):
                            nc.vector.reciprocal(rc, pso[64:65, :])
                        psb = bp.tile([64, 512], F32, tag="bp")
                        nc.tensor.matmul(psb, ones[:, 0:64], rc,
                                         start=True, stop=True)
                        rb = rbpool.tile([64, 512], F32R, tag="rb")
                        nc.vector.tensor_copy(rb, psb)
                        if po == 0:
                            nc.vector.tensor_mul(at2[t][0:64, :],
                                                 pso[0:64, :], rb)
                        else:
                            nrm = nmpool.tile([64, 512], F32R, tag="nrm")
                            nc.vector.tensor_mul(nrm, pso[0:64, :], rb)
                            # partition shift 0-63 -> 64-127 via sbuf-sbuf DMA
                            nc.sync.dma_start(out=at2[t][64:128, :], in_=nrm)

                    for lc in range(4):
                        for no in range(2):
                            psf = fp.tile([128, 512], F32, tag="fp")
                            for pr in range(4):
                                nc.tensor.matmul(
                                    psf,
                                    at2[pr][:, lc * 128:(lc + 1) * 128],
                                    wo_t[pr][:, no * 512:(no + 1) * 512],
                                    start=(pr == 0), stop=False)
                            nc.tensor.matmul(
                                psf, ones,
                                bo_sb[:, no * 512:(no + 1) * 512],
                                start=False, stop=True)
                            ot = otpool.tile([128, 512], F32, tag="ot")
                            nc.vector.tensor_copy(ot, psf)
                            nc.sync.dma_start(
                                out=y[jq * 512 + lc * 128:
                                      jq * 512 + (lc + 1) * 128,
                                      no * 512:(no + 1) * 512],
                                in_=ot)

    nc.finalize()
    return nc


def _make_masks():
    kk = np.arange(128)[:, None]
    qq = np.arange(512)[None, :]
    return np.stack([(qq >= kk + 128 * m) for m in range(4)]
                    ).astype(np.float32)


def make_in_maps(query, key, value, W_packed, b_packed, W_out, b_out):
    query = np.asarray(query, dtype=np.float32)
    key = np.asarray(key, dtype=np.float32)
    value = np.asarray(value, dtype=np.float32)
    W_packed = np.asarray(W_packed, dtype=np.float32)
    b_packed = np.asarray(b_packed, dtype=np.float32)
    W_out = np.asarray(W_out, dtype=np.float32)
    b_out = np.asarray(b_out, dtype=np.float32)

    msk = _make_masks()
    xqT = [np.ascontiguousarray(query[b].T) for b in range(N)]
    xkT = [np.ascontiguousarray(key[b].T) for b in range(N)]
    xvT = [np.ascontiguousarray(value[b].T) for b in range(N)]

    in_maps = []
    for c in range(NCORES):
        b, g = c // 2, c % 2
        sl = slice(g * ES, (g + 1) * ES)
        in_maps.append({
            "xq": xqT[b], "xk": xkT[b], "xv": xvT[b],
            "wq": np.ascontiguousarray(W_packed[0 * E:][:E][sl, :].T),
            "wk": np.ascontiguousarray(W_packed[1 * E:][:E][sl, :].T),
            "wv": np.ascontiguousarray(W_packed[2 * E:][:E][sl, :].T),
            "wo": np.ascontiguousarray(W_out[:, sl].T),
            "bq": np.ascontiguousarray(
                b_packed[0 * E:][:E][sl].reshape(4, 128).T),
            "bk": np.ascontiguousarray(
                b_packed[1 * E:][:E][sl].reshape(4, 128).T),
            "bv": b_packed[2 * E:][:E][sl].reshape(1, ES).copy(),
            "bo": (b_out.reshape(1, E).copy() if g == 0
                   else np.zeros((1, E), np.float32)),
            "msk": msk,
        })
    return in_maps


def get_nc():
    if "nc" not in _CACHE:
        _CACHE["nc"] = _build()
    return _CACHE["nc"]


def kernel(query, key, value, W_packed, b_packed, W_out, b_out):
    nc = get_nc()
    in_maps = make_in_maps(query, key, value, W_packed, b_packed,
                           W_out, b_out)
    res = bass_utils.run_bass_kernel_spmd(nc, in_maps,
                                          core_ids=list(range(NCORES)))
    out = np.stack([res.results[2 * b]["y"] + res.results[2 * b + 1]["y"]
                    for b in range(N)])
    return out.astype(np.float32)


# revision 9
# speedup vs baseline: 1.3786x; 1.3786x over previous
"""Multi-head causal self-attention (N=4, L=2048, E=1024, H=16) on 8 NeuronCores.

Sharding: core c handles batch b = c//2 and head-group g = c%2 (8 heads,
E-slice of 512). Each core computes its QKV projection slice, causal
attention for its 8 heads, and a partial out-projection (E-contraction over
its 512-slice). Host sums the two partials per batch (bias added on g=0).

On-chip layout (per core):
  qT/kT: [e_out(512) x L] transposed activations (4 tiles of [128, 2048])
  v:     [L x e_out] natural layout, per l-block tiles [128, 8 heads, 65]
         (65th column = 1.0 -> the ones column makes the attention matmul
          also produce the softmax denominator as output row 64)
  scores are computed transposed: s^T[l_k, l_q] = k^T.T @ q^T, so the
  av matmul (lhsT = v tile, rhs = exp(s^T)) needs no transposes at all.
  Softmax uses no max-subtraction (scores*scale is O(1) by construction),
  masking is multiplicative post-exp on the block-diagonal tiles only.
All matmuls run as float32r (single-pass, 1 cyc/row at N=512).
"""

import numpy as np

import concourse.bacc as bacc
import concourse.mybir as mybir
import concourse.tile as tile
from concourse import bass_utils

F32 = mybir.dt.float32
F32R = mybir.dt.float32r
AF = mybir.ActivationFunctionType

N, L, E = 4, 2048, 1024
H, EH = 16, 64
NCORES = 8
ES = 512          # e-slice per core (8 heads x 64)
SCALE = 1.0 / np.sqrt(EH)

_CACHE = {}


def _build():
    nc = bacc.Bacc("TRN2", target_bir_lowering=False, debug=False,
                   num_devices=NCORES)
    xq = nc.dram_tensor("xq", (E, L), F32, kind="ExternalInput").ap()
    xk = nc.dram_tensor("xk", (E, L), F32, kind="ExternalInput").ap()
    xv = nc.dram_tensor("xv", (E, L), F32, kind="ExternalInput").ap()
    wq = nc.dram_tensor("wq", (E, ES), F32, kind="ExternalInput").ap()
    wk = nc.dram_tensor("wk", (E, ES), F32, kind="ExternalInput").ap()
    wv = nc.dram_tensor("wv", (E, ES), F32, kind="ExternalInput").ap()
    wo = nc.dram_tensor("wo", (ES, E), F32, kind="ExternalInput").ap()
    bq = nc.dram_tensor("bq", (128, 4), F32, kind="ExternalInput").ap()
    bk = nc.dram_tensor("bk", (128, 4), F32, kind="ExternalInput").ap()
    bv = nc.dram_tensor("bv", (1, ES), F32, kind="ExternalInput").ap()
    bo = nc.dram_tensor("bo", (1, E), F32, kind="ExternalInput").ap()
    msk = nc.dram_tensor("msk", (4, 128, 512), F32, kind="ExternalInput").ap()
    y = nc.dram_tensor("y", (L, E), F32, kind="ExternalOutput").ap()

    with tile.TileContext(nc) as tc:
        with tc.tile_pool(name="const", bufs=1) as cpool, \
             tc.tile_pool(name="qt", bufs=4) as qtpool, \
             tc.tile_pool(name="kt", bufs=4) as ktpool, \
             tc.tile_pool(name="vp", bufs=16) as vpool, \
             tc.tile_pool(name="wo", bufs=4) as wopool:

            ones_st = cpool.tile([1, 128], F32)
            nc.vector.memset(ones_st, 1.0)
            ones = cpool.tile([1, 128], F32R)
            nc.vector.tensor_copy(ones, ones_st)
            vcol = cpool.tile([128, 8], F32)
            nc.vector.memset(vcol, 1.0)
            bq_sb = cpool.tile([128, 4], F32)
            bk_sb = cpool.tile([128, 4], F32)
            bv_sb = cpool.tile([1, ES], F32R)
            bo_sb = cpool.tile([1, E], F32R)
            mask_sb = cpool.tile([128, 4, 512], F32)
            nc.sync.dma_start(out=bq_sb, in_=bq)
            nc.sync.dma_start(out=bk_sb, in_=bk)
            nc.sync.dma_start(out=bv_sb, in_=bv.bitcast(F32R))
            nc.sync.dma_start(out=bo_sb, in_=bo.bitcast(F32R))
            nc.sync.dma_start(out=mask_sb,
                              in_=msk.rearrange("m p q -> p m q"))

            wo_t = []
            for pr in range(4):
                t = wopool.tile([128, E], F32R, tag="wo", name=f"wo{pr}")
                nc.sync.dma_start(
                    out=t, in_=wo[pr * 128:(pr + 1) * 128, :].bitcast(F32R))
                wo_t.append(t)

            qt = [qtpool.tile([128, L], F32R, tag="qt", name=f"qt{i}")
                  for i in range(4)]
            kt = [ktpool.tile([128, L], F32R, tag="kt", name=f"kt{i}")
                  for i in range(4)]
            vts = [vpool.tile([128, 8, 65], F32R, tag="v", name=f"v{i}")
                   for i in range(16)]

            # ---------------- Phase 1: projections ----------------
            with tc.tile_pool(name="wtile", bufs=8) as wpool, \
                 tc.tile_pool(name="xs", bufs=2) as xpool, \
                 tc.tile_pool(name="ps1", bufs=4, space="PSUM") as ps1:

                def load_w(w_dram):
                    ts = []
                    for ko in range(8):
                        t = wpool.tile([128, ES], F32R, tag="w", name=f"w{ko}")
                        nc.sync.dma_start(
                            out=t,
                            in_=w_dram[ko * 128:(ko + 1) * 128, :].bitcast(F32R))
                        ts.append(t)
                    return ts

                def proj_qk(x_dram, w_tiles, bias_sb, out_tiles):
                    for lb in range(4):
                        xt = xpool.tile([128, 8, 512], F32R, tag="x", name="xt")
                        nc.sync.dma_start(
                            out=xt,
                            in_=x_dram.rearrange("(ko ki) l -> ki ko l", ki=128)
                            [:, :, lb * 512:(lb + 1) * 512].bitcast(F32R))
                        for eo in range(4):
                            ps = ps1.tile([128, 512], F32, tag="ps1", name="ps")
                            for ko in range(8):
                                nc.tensor.matmul(
                                    ps,
                                    w_tiles[ko][:, eo * 128:(eo + 1) * 128],
                                    xt[:, ko, :],
                                    start=(ko == 0), stop=(ko == 7))
                            nc.scalar.activation(
                                out_tiles[eo][:, lb * 512:(lb + 1) * 512],
                                ps, AF.Identity,
                                bias=bias_sb[:, eo:eo + 1], scale=1.0)

                wq_t = load_w(wq)
                proj_qk(xq, wq_t, bq_sb, qt)
                wk_t = load_w(wk)
                proj_qk(xk, wk_t, bk_sb, kt)

                wv_t = load_w(wv)
                for lb in range(4):
                    xt = xpool.tile([128, 8, 512], F32R, tag="x", name="xt")
                    nc.sync.dma_start(
                        out=xt,
                        in_=xv.rearrange("(ko ki) l -> ki ko l", ki=128)
                        [:, :, lb * 512:(lb + 1) * 512].bitcast(F32R))
                    for i in range(4):
                        lv = lb * 4 + i
                        ps = ps1.tile([128, 512], F32, tag="ps1", name="ps")
                        for ko in range(8):
                            nc.tensor.matmul(
                                ps, xt[:, ko, i * 128:(i + 1) * 128], wv_t[ko],
                                start=(ko == 0), stop=False)
                        nc.tensor.matmul(ps, ones, bv_sb,
                                         start=False, stop=True)
                        nc.scalar.copy(
                            vts[lv][:, :, 0:64],
                            ps[:, :].rearrange("p (h e) -> p h e", e=64))
                        nc.vector.tensor_copy(vts[lv][:, :, 64], vcol)

            # ---------------- Phase 2+3: attention + out-proj ----------------
            with tc.tile_pool(name="sp", bufs=2, space="PSUM") as sp, \
                 tc.tile_pool(name="op", bufs=2, space="PSUM") as op, \
                 tc.tile_pool(name="bp", bufs=1, space="PSUM") as bp, \
                 tc.tile_pool(name="fp", bufs=1, space="PSUM") as fp, \
                 tc.tile_pool(name="pp", bufs=3) as ppool, \
                 tc.tile_pool(name="tp", bufs=2) as tpool, \
                 tc.tile_pool(name="rp", bufs=2) as rpool, \
                 tc.tile_pool(name="rb", bufs=2) as rbpool, \
                 tc.tile_pool(name="nm", bufs=2) as nmpool, \
                 tc.tile_pool(name="a2", bufs=8) as a2pool, \
                 tc.tile_pool(name="ot", bufs=2) as otpool:

                # Software-pipelined emission: av matmuls trail their scores
                # group by one group, and the normalize / out-proj chains
                # trail by one head, so the in-order PE queue never sits
                # behind a wait on ACT(exp) or DVE outputs.
                from collections import deque
                pending = deque()

                def flush_one():
                    if pending:
                        pending.popleft()()

                def flush_all():
                    while pending:
                        pending.popleft()()

                def mk_av(pso, h, g, nkb, p2):
                    def emit():
                        for i in range(2):
                            kb = 2 * g + i
                            nc.tensor.matmul(
                                pso, vts[kb][:, h, :], p2[:, i, :],
                                start=(kb == 0), stop=(kb == nkb - 1))
                    return emit

                def mk_norm(pso, at2, t, po):
                    def emit():
                        dsb = rpool.tile([1, 512], F32, tag="dsb", name="dsb")
                        nc.vector.tensor_copy(dsb, pso[64:65, :])
                        rcf = rpool.tile([1, 512], F32, tag="rcf", name="rcf")
                        nc.vector.reciprocal_approx_fast(rcf, dsb)
                        rc = rpool.tile([1, 512], F32R, tag="rc", name="rc")
                        nc.vector.tensor_copy(rc, rcf)
                        psb = bp.tile([64, 512], F32, tag="bp", name="psb")
                        nc.tensor.matmul(psb, ones[:, 0:64], rc,
                                         start=True, stop=True)
                        rb = rbpool.tile([64, 512], F32R, tag="rb", name="rb")
                        nc.vector.tensor_copy(rb, psb)
                        if po == 0:
                            nc.vector.tensor_mul(at2[t][0:64, :],
                                                 pso[0:64, :], rb)
                        else:
                            nrm = nmpool.tile([64, 512], F32R, tag="nrm",
                                              name="nrm")
                            nc.vector.tensor_mul(nrm, pso[0:64, :], rb)
                            # partition shift 0-63 -> 64-127 via sbuf-sbuf DMA
                            nc.sync.dma_start(out=at2[t][64:128, :], in_=nrm)
                    return emit

                def mk_outproj(at2, jq):
                    def emit():
                        for lc in range(4):
                            for no in range(2):
                                psf = fp.tile([128, 512], F32, tag="fp",
                                              name="psf")
                                for pr in range(4):
                                    nc.tensor.matmul(
                                        psf,
                                        at2[pr][:, lc * 128:(lc + 1) * 128],
                                        wo_t[pr][:, no * 512:(no + 1) * 512],
                                        start=(pr == 0), stop=False)
                                nc.tensor.matmul(
                                    psf, ones,
                                    bo_sb[:, no * 512:(no + 1) * 512],
                                    start=False, stop=True)
                                ot = otpool.tile([128, 512], F32, tag="ot",
                                                 name="ot")
                                nc.vector.tensor_copy(ot, psf)
                                nc.sync.dma_start(
                                    out=y[jq * 512 + lc * 128:
                                          jq * 512 + (lc + 1) * 128,
                                          no * 512:(no + 1) * 512],
                                    in_=ot)
                    return emit

                for jq in range(4):
                    at2 = [a2pool.tile([128, 512], F32R, tag="a2",
                                       name=f"a2_{i}") for i in range(4)]
                    for h in range(8):
                        t, po = h // 2, (h % 2) * 64
                        pso = op.tile([65, 512], F32, tag="op", name="pso")
                        nkb = 4 * (jq + 1)
                        for g in range(2 * (jq + 1)):
                            pss = sp.tile([128, 2, 512], F32, tag="sp",
                                          name="pss")
                            for i in range(2):
                                kb = 2 * g + i
                                nc.tensor.matmul(
                                    pss[:, i, :],
                                    kt[t][po:po + 64,
                                          kb * 128:(kb + 1) * 128],
                                    qt[t][po:po + 64,
                                          jq * 512:(jq + 1) * 512],
                                    start=True, stop=True)
                            p2 = ppool.tile([128, 2, 512], F32R, tag="p",
                                            name="p2")
                            if g >= 2 * jq:   # block-diagonal: needs mask
                                tmp = tpool.tile([128, 2, 512], F32,
                                                 tag="tmp", name="tmp")
                                nc.scalar.activation(tmp, pss, AF.Exp,
                                                     scale=float(SCALE))
                                mi = 2 * (g - 2 * jq)
                                nc.vector.tensor_mul(
                                    p2, tmp, mask_sb[:, mi:mi + 2, :])
                            else:
                                nc.scalar.activation(p2, pss, AF.Exp,
                                                     scale=float(SCALE))
                            flush_one()
                            pending.append(mk_av(pso, h, g, nkb, p2))
                        pending.append(mk_norm(pso, at2, t, po))
                    pending.append(mk_outproj(at2, jq))
                flush_all()

    nc.finalize()
    return nc


def _make_masks():
    kk = np.arange(128)[:, None]
    qq = np.arange(512)[None, :]
    return np.stack([(qq >= kk + 128 * m) for m in range(4)]
                    ).astype(np.float32)


def make_in_maps(query, key, value, W_packed, b_packed, W_out, b_out):
    query = np.asarray(query, dtype=np.float32)
    key = np.asarray(key, dtype=np.float32)
    value = np.asarray(value, dtype=np.float32)
    W_packed = np.asarray(W_packed, dtype=np.float32)
    b_packed = np.asarray(b_packed, dtype=np.float32)
    W_out = np.asarray(W_out, dtype=np.float32)
    b_out = np.asarray(b_out, dtype=np.float32)

    msk = _make_masks()
    xqT = [np.ascontiguousarray(query[b].T) for b in range(N)]
    xkT = [np.ascontiguousarray(key[b].T) for b in range(N)]
    xvT = [np.ascontiguousarray(value[b].T) for b in range(N)]

    in_maps = []
    for c in range(NCORES):
        b, g = c // 2, c % 2
        sl = slice(g * ES, (g + 1) * ES)
        in_maps.append({
            "xq": xqT[b], "xk": xkT[b], "xv": xvT[b],
            "wq": np.ascontiguousarray(W_packed[0 * E:][:E][sl, :].T),
            "wk": np.ascontiguousarray(W_packed[1 * E:][:E][sl, :].T),
            "wv": np.ascontiguousarray(W_packed[2 * E:][:E][sl, :].T),
            "wo": np.ascontiguousarray(W_out[:, sl].T),
            "bq": np.ascontiguousarray(
                b_packed[0 * E:][:E][sl].reshape(4, 128).T),
            "bk": np.ascontiguousarray(
                b_packed[1 * E:][:E][sl].reshape(4, 128).T),
            "bv": b_packed[2 * E:][:E][sl].reshape(1, ES).copy(),
            "bo": (b_out.reshape(1, E).copy() if g == 0
                   else np.zeros((1, E), np.float32)),
            "msk": msk,
        })
    return in_maps


def get_nc():
    if "nc" not in _CACHE:
        _CACHE["nc"] = _build()
    return _CACHE["nc"]


def kernel(query, key, value, W_packed, b_packed, W_out, b_out):
    nc = get_nc()
    in_maps = make_in_maps(query, key, value, W_packed, b_packed,
                           W_out, b_out)
    res = bass_utils.run_bass_kernel_spmd(nc, in_maps,
                                          core_ids=list(range(NCORES)))
    out = np.stack([res.results[2 * b]["y"] + res.results[2 * b + 1]["y"]
                    for b in range(N)])
    return out.astype(np.float32)
